# revision 6
# baseline (speedup 1.0000x reference)
"""Trainium2 Bass kernel for Mixtral SwiGLU MLP with HQQ 4-bit weights (v2).

Strategy (per core; 8-way tensor-parallel over the intermediate dim):
  - All weights host-dequantized to fp8e4m3; activations split x = x_hi + x_lo
    (both fp8, exact to fp8 rounding) so every matmul runs in fp8 DoubleRow
    mode (K=256/instruction, 0.5 cycles/row) with bf16-level accuracy.
  - h = silu(g)*u is rescaled by 1/S (S=2^14) and stored fp8; the down
    projection uses h_hi only (adds ~1e-3 rel err) and rescales by S on the
    PSUM->SBUF copy. Partial outputs are written bf16 in a packed layout and
    summed on host.
  - PSUM: one pool, 4 tags x 2 bufs = 8 banks; up and down phases share it.
  - DMA: x/w1 on SP HWDGE, x_lo/w3/w2 on Activation HWDGE, output stores on
    Pool SWDGE; weight tiles triple-buffered, w2 prefetched during up phase.
"""

import os
import sys

for _p in ("/opt/trn_rl_repo", "/root/.axon_site/_ro/trn_rl_repo"):
    if os.path.isdir(_p) and _p not in sys.path:
        sys.path.insert(0, _p)

import ml_dtypes
import numpy as np

import concourse.bacc as bacc
import concourse.mybir as mybir
import concourse.tile as tile
from concourse.bass_utils import run_bass_kernel_spmd

BF16 = ml_dtypes.bfloat16
E4M3 = ml_dtypes.float8_e4m3

N_CORES = 8
TOK = 4096
HID = 4096
INT = 14336
GS = 64

INT_SH = INT // N_CORES          # 1792 intermediate rows per core
TS = 1024                        # token super-block
SUPERS = TOK // TS               # 4
I_TILES = INT_SH // 128          # 14
H_TILES = HID // 128             # 32
DP_W = 2048                      # output-column panel width
DPS = HID // DP_W                # 2
S = 16384.0                      # h rescale so h/S fits fp8e4m3 range

_CACHE = {}


def _build_nc(repeats=1):
    key = ("nc", repeats)
    if key in _CACHE:
        return _CACHE[key]

    nc = bacc.Bacc("TRN2", target_bir_lowering=False, debug=False)
    bf = mybir.dt.bfloat16
    f8 = mybir.dt.float8e4
    f32 = mybir.dt.float32
    DR = mybir.MatmulPerfMode.DoubleRow
    Silu = mybir.ActivationFunctionType.Silu
    Copy = mybir.ActivationFunctionType.Copy

    xh_d = nc.dram_tensor("xh", [SUPERS, 128, H_TILES, TS], f8, kind="ExternalInput")
    xl_d = nc.dram_tensor("xl", [SUPERS, 128, H_TILES, TS], f8, kind="ExternalInput")
    w1_d = nc.dram_tensor("w1t", [I_TILES, 128, H_TILES, 128], f8, kind="ExternalInput")
    w3_d = nc.dram_tensor("w3t", [I_TILES, 128, H_TILES, 128], f8, kind="ExternalInput")
    w2_d = nc.dram_tensor("w2t", [DPS, 128, I_TILES, DP_W], f8, kind="ExternalInput")
    # packed partial output: [sb, tt, dp, s, token 128, col 512] bf16
    out_d = nc.dram_tensor("out", [SUPERS, TS // 128, DPS, 4, 128, 512], bf,
                           kind="ExternalOutput")

    with tile.TileContext(nc) as tc:
        with (
            tc.tile_pool(name="xp", bufs=1) as xp,
            tc.tile_pool(name="wp", bufs=3) as wp,
            tc.tile_pool(name="w2p", bufs=2) as w2p,
            tc.tile_pool(name="hp", bufs=1) as hp,
            tc.tile_pool(name="tp", bufs=2) as tp,
            tc.tile_pool(name="op", bufs=4) as op,
            tc.tile_pool(name="ps", bufs=2, space="PSUM") as ps,
        ):
            for sb in [s_ for _ in range(repeats) for s_ in range(SUPERS)]:
                xh_sb = xp.tile([128, H_TILES, TS], f8, tag="xh")
                xl_sb = xp.tile([128, H_TILES, TS], f8, tag="xl")
                for q in range(4):
                    sl = slice(q * 8, (q + 1) * 8)
                    nc.sync.dma_start(xh_sb[:, sl, :], xh_d[sb, :, sl, :])
                    nc.scalar.dma_start(xl_sb[:, sl, :], xl_d[sb, :, sl, :])

                # prefetch the first two w2 tiles during the up phase
                w2_tiles = {}
                for dp in range(2):
                    w2_tiles[dp] = w2p.tile([128, I_TILES, DP_W], f8, tag="w2",
                                            name=f"w2_{sb}_{dp}")
                    nc.scalar.dma_start(w2_tiles[dp][:], w2_d[dp])

                h_sb = hp.tile([128, I_TILES, TS], f8, tag="h")

                for it in range(I_TILES):
                    w1_sb = wp.tile([128, H_TILES, 128], f8, tag="w1")
                    nc.sync.dma_start(w1_sb[:], w1_d[it])
                    w3_sb = wp.tile([128, H_TILES, 128], f8, tag="w3")
                    nc.scalar.dma_start(w3_sb[:], w3_d[it])

                    g0 = ps.tile([128, 512], f32, tag="g0")
                    g1 = ps.tile([128, 512], f32, tag="g1")
                    u0 = ps.tile([128, 512], f32, tag="u0")
                    u1 = ps.tile([128, 512], f32, tag="u1")
                    c0, c1 = slice(0, 512), slice(512, 1024)
                    for j, src in enumerate((xh_sb, xl_sb)):
                        for q in range(H_TILES // 2):
                            kp = slice(2 * q, 2 * q + 2)
                            st = (j == 0 and q == 0)
                            sp = (j == 1 and q == H_TILES // 2 - 1)
                            w1t_ = w1_sb[:, kp, :]
                            w3t_ = w3_sb[:, kp, :]
                            nc.tensor.matmul(g0[:], w1t_, src[:, kp, c0],
                                             start=st, stop=sp, perf_mode=DR)
                            nc.tensor.matmul(g1[:], w1t_, src[:, kp, c1],
                                             start=st, stop=sp, perf_mode=DR)
                            nc.tensor.matmul(u0[:], w3t_, src[:, kp, c0],
                                             start=st, stop=sp, perf_mode=DR)
                            nc.tensor.matmul(u1[:], w3t_, src[:, kp, c1],
                                             start=st, stop=sp, perf_mode=DR)
                    for half, (g, u) in enumerate(((g0, u0), (g1, u1))):
                        cols = slice(half * 512, (half + 1) * 512)
                        sil = tp.tile([128, 512], bf, tag="sil")
                        nc.scalar.activation(sil[:], g[:], Silu)
                        u_s = tp.tile([128, 512], bf, tag="us")
                        nc.scalar.activation(u_s[:], u[:], Copy, scale=1.0 / S)
                        nc.vector.tensor_mul(h_sb[:, it, cols], sil[:], u_s[:])

                for dp in range(DPS):
                    w2_sb = w2_tiles[dp]
                    for tt in range(TS // 128):
                        trows = slice(tt * 128, (tt + 1) * 128)
                        obanks = [ps.tile([128, 512], f32, tag=t,
                                          name=f"o{i}_{dp}_{tt}")
                                  for i, t in enumerate(("g0", "u0", "g1", "u1"))]
                        for k in range(I_TILES // 2):
                            kp = slice(2 * k, 2 * k + 2)
                            st = (k == 0)
                            sp = (k == I_TILES // 2 - 1)
                            ht = h_sb[:, kp, trows]
                            for s_ in range(4):
                                nc.tensor.matmul(
                                    obanks[s_][:], ht,
                                    w2_sb[:, kp, s_ * 512:(s_ + 1) * 512],
                                    start=st, stop=sp, perf_mode=DR)
                        for s_, o in enumerate(obanks):
                            ot = op.tile([128, 512], bf, tag="ot")
                            if s_ % 2 == 0:
                                nc.scalar.activation(ot[:], o[:], Copy, scale=S)
                            else:
                                nc.vector.tensor_scalar_mul(ot[:], o[:], S)
                            nc.gpsimd.dma_start(out_d[sb, tt, dp, s_], ot[:])

    nc.compile()
    _CACHE[key] = nc
    return nc


def _dequant(q, s, z):
    out, inp = q.shape
    g = inp // GS
    qf = np.asarray(q, np.float32).reshape(out, g, GS)
    w = (qf - np.asarray(z, np.float32)[:, :, None]) * \
        np.asarray(s, np.float32)[:, :, None]
    return w.reshape(out, inp)


def _prep_in_maps(hidden_states, w1_q, w1_scale, w1_zero, w3_q, w3_scale,
                  w3_zero, w2_q, w2_scale, w2_zero):
    x = np.asarray(hidden_states, np.float32)
    x_hi = x.astype(E4M3)
    x_lo = (x - x_hi.astype(np.float32)).astype(E4M3)

    def xperm(a):
        # [sb, p, a, t] = x[sb*TS + t, a*128 + p]
        return np.ascontiguousarray(
            a.reshape(SUPERS, TS, H_TILES, 128).transpose(0, 3, 2, 1))

    xh = xperm(x_hi)
    xl = xperm(x_lo)

    def up_shard(q, s, z, c):
        rows = slice(c * INT_SH, (c + 1) * INT_SH)
        wd = _dequant(q[rows], s[rows], z[rows]).astype(E4M3)
        return np.ascontiguousarray(
            wd.reshape(I_TILES, 128, H_TILES, 128).transpose(0, 3, 2, 1))

    def down_shard(q, s, z, c):
        cols = slice(c * INT_SH, (c + 1) * INT_SH)
        gsl = slice(c * (INT_SH // GS), (c + 1) * (INT_SH // GS))
        wd = _dequant(np.ascontiguousarray(q[:, cols]), s[:, gsl],
                      z[:, gsl]).astype(E4M3)                    # [HID, INT_SH]
        return np.ascontiguousarray(
            wd.reshape(DPS, DP_W, I_TILES, 128).transpose(0, 3, 2, 1))

    in_maps = []
    for c in range(N_CORES):
        in_maps.append({
            "xh": xh,
            "xl": xl,
            "w1t": up_shard(w1_q, w1_scale, w1_zero, c),
            "w3t": up_shard(w3_q, w3_scale, w3_zero, c),
            "w2t": down_shard(w2_q, w2_scale, w2_zero, c),
        })
    return in_maps


def kernel(**inputs):
    nc = _build_nc()
    in_maps = _prep_in_maps(**inputs)
    res = run_bass_kernel_spmd(nc, in_maps, core_ids=list(range(N_CORES)))
    acc = np.zeros((SUPERS, TS // 128, DPS, 4, 128, 512), np.float32)
    for c in range(N_CORES):
        acc += res.results[c]["out"].astype(np.float32)
    # [sb, tt, dp, s, r, c] -> [sb*1024 + tt*128 + r, dp*1024 + s*512 + c]
    out = acc.transpose(0, 1, 4, 2, 3, 5).reshape(TOK, HID)
    return np.ascontiguousarray(out)


if __name__ == "__main__":
    rng = np.random.default_rng(0)
    ins = {
        "hidden_states": rng.standard_normal((TOK, HID)).astype(np.float32),
        "w1_q": rng.integers(0, 16, (INT, HID)).astype(np.int32),
        "w1_scale": rng.random((INT, HID // GS)).astype(np.float32),
        "w1_zero": rng.random((INT, HID // GS)).astype(np.float32),
        "w3_q": rng.integers(0, 16, (INT, HID)).astype(np.int32),
        "w3_scale": rng.random((INT, HID // GS)).astype(np.float32),
        "w3_zero": rng.random((INT, HID // GS)).astype(np.float32),
        "w2_q": rng.integers(0, 16, (HID, INT)).astype(np.int32),
        "w2_scale": rng.random((HID, INT // GS)).astype(np.float32),
        "w2_zero": rng.random((HID, INT // GS)).astype(np.float32),
    }
    out = kernel(**ins)
    print("out", out.shape, out.dtype, float(np.abs(out).max()))


# revision 7
# speedup vs baseline: 6.7453x; 6.7453x over previous
"""Trainium2 Bass kernel for Mixtral SwiGLU MLP with HQQ 4-bit weights (v2).

Strategy (per core; 8-way tensor-parallel over the intermediate dim):
  - All weights host-dequantized to fp8e4m3; activations split x = x_hi + x_lo
    (both fp8, exact to fp8 rounding) so every matmul runs in fp8 DoubleRow
    mode (K=256/instruction, 0.5 cycles/row) with bf16-level accuracy.
  - h = silu(g)*u is rescaled by 1/S (S=2^14) and stored fp8; the down
    projection uses h_hi only (adds ~1e-3 rel err) and rescales by S on the
    PSUM->SBUF copy. Partial outputs are written bf16 in a packed layout and
    summed on host.
  - PSUM: one pool, 4 tags x 2 bufs = 8 banks; up and down phases share it.
  - DMA: x/w1 on SP HWDGE, x_lo/w3/w2 on Activation HWDGE, output stores on
    Pool SWDGE; weight tiles triple-buffered, w2 prefetched during up phase.
"""

import os
import sys

for _p in ("/opt/trn_rl_repo", "/root/.axon_site/_ro/trn_rl_repo"):
    if os.path.isdir(_p) and _p not in sys.path:
        sys.path.insert(0, _p)

import ml_dtypes
import numpy as np

import concourse.bacc as bacc
import concourse.mybir as mybir
import concourse.tile as tile
from concourse.bass_utils import run_bass_kernel_spmd

BF16 = ml_dtypes.bfloat16
E4M3 = ml_dtypes.float8_e4m3

N_CORES = 8
TOK = 4096
HID = 4096
INT = 14336
GS = 64

INT_SH = INT // N_CORES          # 1792 intermediate rows per core
TS = 1024                        # token super-block
SUPERS = TOK // TS               # 4
I_TILES = INT_SH // 128          # 14
H_TILES = HID // 128             # 32
DP_W = 2048                      # output-column panel width
DPS = HID // DP_W                # 2
S = 16384.0                      # h rescale so h/S fits fp8e4m3 range

_CACHE = {}


def _build_nc(repeats=1):
    key = ("nc", repeats)
    if key in _CACHE:
        return _CACHE[key]

    nc = bacc.Bacc("TRN2", target_bir_lowering=False, debug=False)
    bf = mybir.dt.bfloat16
    f8 = mybir.dt.float8e4
    f32 = mybir.dt.float32
    DR = mybir.MatmulPerfMode.DoubleRow
    Silu = mybir.ActivationFunctionType.Silu
    Copy = mybir.ActivationFunctionType.Copy

    xh_d = nc.dram_tensor("xh", [SUPERS, 128, H_TILES, TS], f8, kind="ExternalInput")
    xl_d = nc.dram_tensor("xl", [SUPERS, 128, H_TILES, TS], f8, kind="ExternalInput")
    w1_d = nc.dram_tensor("w1t", [I_TILES, 128, H_TILES, 128], f8, kind="ExternalInput")
    w3_d = nc.dram_tensor("w3t", [I_TILES, 128, H_TILES, 128], f8, kind="ExternalInput")
    w2_d = nc.dram_tensor("w2t", [DPS, 128, I_TILES, DP_W], f8, kind="ExternalInput")
    # packed partial output: [sb, tt, dp, s, token 128, col 512] bf16
    out_d = nc.dram_tensor("out", [SUPERS, TS // 128, DPS, 4, 128, 512], bf,
                           kind="ExternalOutput")

    with tile.TileContext(nc) as tc:
        with (
            tc.tile_pool(name="xp", bufs=1) as xp,
            tc.tile_pool(name="wp", bufs=3) as wp,
            tc.tile_pool(name="w2p", bufs=2) as w2p,
            tc.tile_pool(name="hp", bufs=1) as hp,
            tc.tile_pool(name="tp", bufs=2) as tp,
            tc.tile_pool(name="op", bufs=4) as op,
            tc.tile_pool(name="ps", bufs=2, space="PSUM") as ps,
        ):
            for sb in [s_ for _ in range(repeats) for s_ in range(SUPERS)]:
                xh_sb = xp.tile([128, H_TILES, TS], f8, tag="xh")
                xl_sb = xp.tile([128, H_TILES, TS], f8, tag="xl")
                for q in range(4):
                    sl = slice(q * 8, (q + 1) * 8)
                    nc.sync.dma_start(xh_sb[:, sl, :], xh_d[sb, :, sl, :])
                    nc.scalar.dma_start(xl_sb[:, sl, :], xl_d[sb, :, sl, :])

                # prefetch the first two w2 tiles during the up phase
                w2_tiles = {}
                for dp in range(2):
                    w2_tiles[dp] = w2p.tile([128, I_TILES, DP_W], f8, tag="w2",
                                            name=f"w2_{sb}_{dp}")
                    nc.scalar.dma_start(w2_tiles[dp][:], w2_d[dp])

                h_sb = hp.tile([128, I_TILES, TS], f8, tag="h")

                for it in range(I_TILES):
                    w1_sb = wp.tile([128, H_TILES, 128], f8, tag="w1")
                    nc.sync.dma_start(w1_sb[:], w1_d[it])
                    w3_sb = wp.tile([128, H_TILES, 128], f8, tag="w3")
                    nc.scalar.dma_start(w3_sb[:], w3_d[it])

                    g0 = ps.tile([128, 512], f32, tag="g0")
                    g1 = ps.tile([128, 512], f32, tag="g1")
                    u0 = ps.tile([128, 512], f32, tag="u0")
                    u1 = ps.tile([128, 512], f32, tag="u1")
                    c0, c1 = slice(0, 512), slice(512, 1024)
                    for q in range(H_TILES // 2):
                        kp = slice(2 * q, 2 * q + 2)
                        st = (q == 0)
                        sp = (q == H_TILES // 2 - 1)
                        w = w1_sb[:, kp, :]
                        nc.tensor.matmul(g0[:], w, xh_sb[:, kp, c0],
                                         start=st, stop=False, perf_mode=DR)
                        nc.tensor.matmul(g1[:], w, xh_sb[:, kp, c1],
                                         start=st, stop=False, perf_mode=DR)
                        nc.tensor.matmul(g0[:], w, xl_sb[:, kp, c0],
                                         start=False, stop=sp, perf_mode=DR)
                        nc.tensor.matmul(g1[:], w, xl_sb[:, kp, c1],
                                         start=False, stop=sp, perf_mode=DR)
                    for q in range(H_TILES // 2):
                        kp = slice(2 * q, 2 * q + 2)
                        st = (q == 0)
                        sp = (q == H_TILES // 2 - 1)
                        w = w3_sb[:, kp, :]
                        nc.tensor.matmul(u0[:], w, xh_sb[:, kp, c0],
                                         start=st, stop=False, perf_mode=DR)
                        nc.tensor.matmul(u1[:], w, xh_sb[:, kp, c1],
                                         start=st, stop=False, perf_mode=DR)
                        nc.tensor.matmul(u0[:], w, xl_sb[:, kp, c0],
                                         start=False, stop=sp, perf_mode=DR)
                        nc.tensor.matmul(u1[:], w, xl_sb[:, kp, c1],
                                         start=False, stop=sp, perf_mode=DR)
                    for half, (g, u) in enumerate(((g0, u0), (g1, u1))):
                        cols = slice(half * 512, (half + 1) * 512)
                        sil = tp.tile([128, 512], bf, tag="sil")
                        nc.scalar.activation(sil[:], g[:], Silu)
                        u_s = tp.tile([128, 512], bf, tag="us")
                        nc.scalar.activation(u_s[:], u[:], Copy, scale=1.0 / S)
                        nc.vector.tensor_mul(h_sb[:, it, cols], sil[:], u_s[:])

                for dp in range(DPS):
                    w2_sb = w2_tiles[dp]
                    for tt in range(TS // 128):
                        trows = slice(tt * 128, (tt + 1) * 128)
                        obanks = [ps.tile([128, 512], f32, tag=t,
                                          name=f"o{i}_{dp}_{tt}")
                                  for i, t in enumerate(("g0", "u0", "g1", "u1"))]
                        for k in range(I_TILES // 2):
                            kp = slice(2 * k, 2 * k + 2)
                            st = (k == 0)
                            sp = (k == I_TILES // 2 - 1)
                            ht = h_sb[:, kp, trows]
                            for s_ in range(4):
                                nc.tensor.matmul(
                                    obanks[s_][:], ht,
                                    w2_sb[:, kp, s_ * 512:(s_ + 1) * 512],
                                    start=st, stop=sp, perf_mode=DR)
                        for s_, o in enumerate(obanks):
                            ot = op.tile([128, 512], bf, tag="ot")
                            if s_ % 2 == 0:
                                nc.scalar.activation(ot[:], o[:], Copy, scale=S)
                            else:
                                nc.vector.tensor_scalar_mul(ot[:], o[:], S)
                            nc.gpsimd.dma_start(out_d[sb, tt, dp, s_], ot[:])

    nc.compile()
    _CACHE[key] = nc
    return nc


def _dequant(q, s, z):
    out, inp = q.shape
    g = inp // GS
    qf = np.asarray(q, np.float32).reshape(out, g, GS)
    w = (qf - np.asarray(z, np.float32)[:, :, None]) * \
        np.asarray(s, np.float32)[:, :, None]
    return w.reshape(out, inp)


def _prep_in_maps(hidden_states, w1_q, w1_scale, w1_zero, w3_q, w3_scale,
                  w3_zero, w2_q, w2_scale, w2_zero):
    x = np.asarray(hidden_states, np.float32)
    x_hi = x.astype(E4M3)
    x_lo = (x - x_hi.astype(np.float32)).astype(E4M3)

    def xperm(a):
        # [sb, p, a, t] = x[sb*TS + t, a*128 + p]
        return np.ascontiguousarray(
            a.reshape(SUPERS, TS, H_TILES, 128).transpose(0, 3, 2, 1))

    xh = xperm(x_hi)
    xl = xperm(x_lo)

    def up_shard(q, s, z, c):
        rows = slice(c * INT_SH, (c + 1) * INT_SH)
        wd = _dequant(q[rows], s[rows], z[rows]).astype(E4M3)
        return np.ascontiguousarray(
            wd.reshape(I_TILES, 128, H_TILES, 128).transpose(0, 3, 2, 1))

    def down_shard(q, s, z, c):
        cols = slice(c * INT_SH, (c + 1) * INT_SH)
        gsl = slice(c * (INT_SH // GS), (c + 1) * (INT_SH // GS))
        wd = _dequant(np.ascontiguousarray(q[:, cols]), s[:, gsl],
                      z[:, gsl]).astype(E4M3)                    # [HID, INT_SH]
        return np.ascontiguousarray(
            wd.reshape(DPS, DP_W, I_TILES, 128).transpose(0, 3, 2, 1))

    in_maps = []
    for c in range(N_CORES):
        in_maps.append({
            "xh": xh,
            "xl": xl,
            "w1t": up_shard(w1_q, w1_scale, w1_zero, c),
            "w3t": up_shard(w3_q, w3_scale, w3_zero, c),
            "w2t": down_shard(w2_q, w2_scale, w2_zero, c),
        })
    return in_maps


def kernel(**inputs):
    nc = _build_nc()
    in_maps = _prep_in_maps(**inputs)
    res = run_bass_kernel_spmd(nc, in_maps, core_ids=list(range(N_CORES)))
    acc = np.zeros((SUPERS, TS // 128, DPS, 4, 128, 512), np.float32)
    for c in range(N_CORES):
        acc += res.results[c]["out"].astype(np.float32)
    # [sb, tt, dp, s, r, c] -> [sb*1024 + tt*128 + r, dp*1024 + s*512 + c]
    out = acc.transpose(0, 1, 4, 2, 3, 5).reshape(TOK, HID)
    return np.ascontiguousarray(out)


if __name__ == "__main__":
    rng = np.random.default_rng(0)
    ins = {
        "hidden_states": rng.standard_normal((TOK, HID)).astype(np.float32),
        "w1_q": rng.integers(0, 16, (INT, HID)).astype(np.int32),
        "w1_scale": rng.random((INT, HID // GS)).astype(np.float32),
        "w1_zero": rng.random((INT, HID // GS)).astype(np.float32),
        "w3_q": rng.integers(0, 16, (INT, HID)).astype(np.int32),
        "w3_scale": rng.random((INT, HID // GS)).astype(np.float32),
        "w3_zero": rng.random((INT, HID // GS)).astype(np.float32),
        "w2_q": rng.integers(0, 16, (HID, INT)).astype(np.int32),
        "w2_scale": rng.random((HID, INT // GS)).astype(np.float32),
        "w2_zero": rng.random((HID, INT // GS)).astype(np.float32),
    }
    out = kernel(**ins)
    print("out", out.shape, out.dtype, float(np.abs(out).max()))
